# revision 33
# baseline (speedup 1.0000x reference)
"""Distributed multi-head causal attention for 8 TRN2 NeuronCores.

Problem: residual [2, 2048, 2048] f32 -> (residual, attn_out [2, 2048, 2048])
  q/k/v = residual @ W_{Q,K,V} + b  (16 heads, d_head 128)
  scores = q k^T / sqrt(128), causal mask, softmax
  out = (pattern @ v) @ W_O + b_O

Sharding: tensor-parallel over heads. Core c computes QKV projections and
attention for heads 2c, 2c+1 over both batches, producing z^T (the
pre-output-projection activations). Two 8-core AllToAlls (one per local
head) redistribute z^T from head-sharded to position-sharded: shard j covers
positions [512*j, 512*(j+1)) of the flattened [batch*seq] axis. After the
A2A each core holds all 16 heads for its own 512 positions and computes the
output projection for just those rows. The host concatenates the 8 shards.

All matmuls in bf16 (inputs pre-cast and pre-packed on host so every load is
one large contiguous DMA), accumulation f32 in PSUM:
  Q^T/K^T [dh, pos] = W^T X^T    (lhsT = W [model, dh], rhs = X^T)
  V [pos, dh*hpc]   = X W_V      (lhsT = X^T tile,      rhs = W_V heads)
  S^T [k, q]        = K Q^T      (lhsT = K^T tile,      rhs = Q^T)
  z^T [dh, q]       = V^T P^T    (lhsT = V tile,        rhs = P^T = exp(S^T))
  den [*, q]        = J acc      (lhsT = all-ones,      rhs = sum_k P^T)
  out [pos, m]      = z W_O      (lhsT = z^T tile,      rhs = W_O)

The emission driver software-pipelines the phases on the single PE queue:
attention chunks (h, b, qc) become runnable as soon as phase 1 has produced
q/k/v for (b, <=qc) (causality: chunk qc only attends keys < (qc+1)*512), so
attention matmuls interleave with later projection units and the attention
phase's scalar-engine exps and vector-engine denominator accumulations hide
under projection PE work. The AV pass is one full-width [128, 512] matmul
per k-tile; softmax denominators come from vector adds of the exp panels
(acc = sum_k P^T) reduced+broadcast by a single all-ones matmul per chunk.
Pass E of the output projection (first-A2A heads) starts the moment the last
attention matmul retires; pass O adds the second-A2A heads.

v2 schedule refinements: startup loads spread over four DMA rings (vector
joins) with wq-h0 split so the first real matmul starts ~10us earlier; W_O
chunks 0/1 load into a dedicated static right after the first A2A trigger;
zf h0 loads ride gpsimd between the two A2A triggers; output stores are
tile-major contiguous 128KB writes (host reassembles) so the kernel-end
barrier no longer waits ~10us on strided output DMA drain.
"""

import numpy as np
import ml_dtypes

import concourse.bass as bass
import concourse.tile as tile
from concourse import bacc, mybir
from concourse.bass_utils import run_bass_kernel_spmd
from concourse.tile_rust import add_dep_helper

BF16 = mybir.dt.bfloat16
F32 = mybir.dt.float32
NP_BF16 = ml_dtypes.bfloat16

FULL = dict(n_heads=16, d_model=2048, d_head=128, batch=2, seq=2048, n_cores=8)
ATTN_SCALE = float(np.sqrt(128.0))


def _derived(cfg):
    d = dict(cfg)
    d["hpc"] = d["n_heads"] // d["n_cores"]             # heads per core
    d["rows"] = d["batch"] * d["seq"] // d["n_cores"]   # out rows per core
    d["qc_size"] = d["rows"]                            # q-chunk == A2A shard
    assert d["qc_size"] <= 512
    d["n_qc"] = d["seq"] // d["qc_size"]                # q chunks per batch
    d["n_kb"] = d["seq"] // 128                         # k blocks per batch
    d["n_mb"] = d["d_model"] // 128                     # model-dim blocks
    d["n_dg"] = d["qc_size"] // 128                     # diag offsets per chunk
    d["n_mc"] = d["d_model"] // 512                     # out m-chunks
    d["n_pb"] = d["rows"] // 128                        # out pos-blocks
    assert d["n_qc"] * d["batch"] == d["n_cores"]
    assert d["d_head"] == 128
    return d


def build_graph(cfg=FULL, enable_asserts=False):
    c = _derived(cfg)
    hpc, QC = c["hpc"], c["qc_size"]
    n_qc, n_kb, n_mb, n_dg = c["n_qc"], c["n_kb"], c["n_mb"], c["n_dg"]
    n_mc, n_pb, rows = c["n_mc"], c["n_pb"], c["rows"]
    n_heads, d_model, seq = c["n_heads"], c["d_model"], c["seq"]
    batch, n_cores = c["batch"], c["n_cores"]
    dpb = QC // 128
    MC = 512

    nc = bacc.Bacc("TRN2", target_bir_lowering=False, debug=False,
                   enable_asserts=enable_asserts, num_devices=n_cores)

    # all inputs pre-packed on host into [128, ...] partition-major layouts
    xt_d = nc.dram_tensor("xt", [128, batch, n_qc, n_mb, QC], BF16,
                          kind="ExternalInput")
    wq_d = nc.dram_tensor("wq", [128, hpc, n_mb, 128], BF16, kind="ExternalInput")
    wk_d = nc.dram_tensor("wk", [128, hpc, n_mb, 128], BF16, kind="ExternalInput")
    wv_d = nc.dram_tensor("wv", [128, n_mb, hpc * 128], BF16, kind="ExternalInput")
    # chunk-major so each 2MB W_O chunk load is fully contiguous
    wo_d = nc.dram_tensor("wo", [n_mc, 128, n_heads, MC], BF16,
                          kind="ExternalInput")
    bq_d = nc.dram_tensor("bq", [128, hpc], F32, kind="ExternalInput")
    bk_d = nc.dram_tensor("bk", [128, hpc], F32, kind="ExternalInput")
    bv_d = nc.dram_tensor("bv", [hpc * 128], F32, kind="ExternalInput")
    mk_d = nc.dram_tensor("mk", [128, n_dg, QC], BF16, kind="ExternalInput")
    # bf16 output halves the final DMA tail; the host adds b_O in f32.
    # tile-major layout so every output store is one contiguous 128KB write
    # (row-major [rows, d_model] made the final DMAs 1KB-segment strided and
    # the kernel-end barrier waited ~10us for them to drain)
    out_d = nc.dram_tensor("out", [n_pb, n_mc, 128, MC], BF16,
                           kind="ExternalOutput")

    rg = [list(range(n_cores))]
    Exp = mybir.ActivationFunctionType.Exp

    with tile.TileContext(nc) as tc:
        with (
            tc.tile_pool(name="stat", bufs=1) as stat,
            tc.tile_pool(name="xin", bufs=2) as xin,
            tc.tile_pool(name="work", bufs=3) as work,
            tc.tile_pool(name="ps", bufs=2, space="PSUM") as ps,
            tc.tile_pool(name="dram", bufs=1, space="DRAM") as dram,
        ):
            wq_sb = stat.tile([128, hpc, n_mb, 128], BF16)
            wk_sb = stat.tile([128, hpc, n_mb, 128], BF16)
            wv_sb = stat.tile([128, n_mb, hpc * 128], BF16, tag="wvzf")
            qt_sb = stat.tile([128, batch, hpc, seq], BF16)
            kt_sb = stat.tile([128, batch, hpc, seq], BF16)
            v_sb = stat.tile([128, batch, n_kb, hpc, 128], BF16)
            # W_O chunks 0/1 get a dedicated static so their 2MB loads can
            # ride the sync ring mid-phase-1 (it idles once the xt stream
            # ends) instead of queueing behind the last attention ships
            wo01_sb = stat.tile([128, 2, n_heads, MC], BF16)
            bq_sb = stat.tile([128, hpc], F32)
            bk_sb = stat.tile([128, hpc], F32)
            vb_sb = stat.tile([128, hpc * 128], F32)
            mk_sb = stat.tile([128, n_dg, QC], BF16)
            ones_sb = stat.tile([128, 128], BF16)
            # h-major so each A2A's zf tiles land with ONE strided DMA
            zf_sb = stat.tile([128, hpc, n_cores, rows], BF16, tag="wvzf")

            a2a_in = [dram.tile([n_cores, 128, rows], BF16, name=f"a2ai{h}")
                      for h in range(hpc)]
            a2a_out = [dram.tile([n_cores, 128, rows], BF16, name=f"a2ao{h}")
                       for h in range(hpc)]
            # the SECOND A2A is split into two position-halves so pass O's
            # first pb tiles can start on half the payload ~transfer-time
            # earlier while the other half is still in flight
            HQ = QC // 2
            a2a_in1 = [dram.tile([n_cores, 128, HQ], BF16, name=f"a2ai1{p}")
                       for p in range(2)]
            a2a_out1 = [dram.tile([n_cores, 128, HQ], BF16, name=f"a2ao1{p}")
                        for p in range(2)]

            # warmup memsets lead the vector queue so the dummy matmuls can
            # start the p-state ramp immediately after the engine barrier
            grb_sb = stat.tile([128, QC], BF16)
            nc.vector.memset(ones_sb, 1.0)
            nc.vector.memset(grb_sb[0:1, 0:1], 1.0)

            # startup loads spread over the three DMA-capable rings
            # (scalar/sync/gpsimd) in exact first-consumption order; wq h0
            # and wv split in halves so the first chains start as soon as
            # their leading mb blocks land instead of waiting for whole
            # weight tensors
            # big weights all on the scalar HWDGE queue as single descriptors
            # (issue cost ~0.6us each), ordered to match first consumption
            # (Q h0, K h0, V, Q h1, K h1); small consts on gpsimd; xt chunks
            # stream on sync alone so chunk N+1 is never stuck behind a
            # weight load. NOTE: splitting the weight loads into halves was
            # tried (v4) and is SLOWER — it halves the strided DMA segment
            # size (4KB -> 2KB) and ring bandwidth drops more than the
            # earlier start gains.
            xt00 = xin.tile([128, n_mb, QC], BF16, tag="xt", name="xt0_0")
            nc.scalar.dma_start(out=wq_sb[:, 0], in_=wq_d[:, 0])
            nc.sync.dma_start(out=xt00[:, 0:4], in_=xt_d[:, 0, 0, 0:4])
            nc.scalar.dma_start(out=xt00[:, 4:8], in_=xt_d[:, 0, 0, 4:8])
            nc.sync.dma_start(out=xt00[:, 8:12], in_=xt_d[:, 0, 0, 8:12])
            nc.scalar.dma_start(out=wv_sb, in_=wv_d[:])
            nc.sync.dma_start(out=xt00[:, 12:16], in_=xt_d[:, 0, 0, 12:16])
            for h in range(1, hpc):
                nc.scalar.dma_start(out=wq_sb[:, h], in_=wq_d[:, h])
            nc.gpsimd.dma_start(out=bq_sb, in_=bq_d[:])
            nc.gpsimd.dma_start(out=bk_sb, in_=bk_d[:])
            # K weights ride the otherwise-idle gpsimd ring so the first K
            # chains never queue behind the Q/V loads on scalar
            nc.gpsimd.dma_start(out=wk_sb, in_=wk_d[:])
            bv_ap = bv_d.ap()
            nc.gpsimd.dma_start(
                out=vb_sb,
                in_=bass.AP(tensor=bv_ap.tensor, offset=bv_ap.offset,
                            ap=[[0, 128]] + list(bv_ap.ap)))
            nc.gpsimd.dma_start(out=mk_sb, in_=mk_d[:])

            # PE warmup: full-width dummy matmuls bridge the gap between
            # the engine barrier and the first weight arrival so the p-state
            # ramp (needs ~3us of continuous HIGH-utilization busy to reach
            # max clock) completes on dummy work. The tail of the warmup is
            # 256-row dummies: fine-grained so a late-arriving first weight
            # is delayed at most ~130ns, but enough of them to keep the
            # clock ramped through the ~+21us DMA-bound start.
            for w in range(16):
                dmy = ps.tile([128, QC], F32, tag="A", name=f"wrm{w}")
                nc.tensor.matmul(dmy, ones_sb, grb_sb,
                                 start=True, stop=True)
            for w in range(36):
                dmy = ps.tile([128, QC // 2], F32, tag="A",
                              name=f"wrs{w}")
                nc.tensor.matmul(dmy, ones_sb, grb_sb[:, 0:QC // 2],
                                 start=True, stop=True)

            wo_tiles = {}
            last_exp = [None]
            markers = set()
            ship_n = [0] * hpc

            def emit_cc(h):
                if h == 0:
                    nc.gpsimd.collective_compute(
                        "AllToAll", mybir.AluOpType.bypass, replica_groups=rg,
                        ins=[a2a_in[0].opt()], outs=[a2a_out[0].opt()])
                else:
                    for p in range(2):
                        nc.gpsimd.collective_compute(
                            "AllToAll", mybir.AluOpType.bypass,
                            replica_groups=rg,
                            ins=[a2a_in1[p].opt()], outs=[a2a_out1[p].opt()])


            def load_zf(h, engs):
                # 8 fully-contiguous 128KB loads (a strided single-descriptor
                # variant measured 55GB/s vs ~350 contiguous), spread across
                # the given rings
                for i in range(n_cores):
                    eng = engs[i % len(engs)]
                    d = eng.dma_start(out=zf_sb[:, h, i, :],
                                      in_=a2a_out[h][i])
                    if eng is nc.scalar and last_exp[0] is not None:
                        # scalar ring also runs the exps: pin after the final
                        # exp so the scheduler cannot hoist the collective-
                        # completion wait into the middle of the attention
                        add_dep_helper(d.ins, last_exp[0].ins,
                                       reason="zf load after attention exps")

            # ---- stream 1: phase-1 QKV projection units ----
            # group order [Q h0, K h0, V, Q h1, K h1]: the ("v", b, qc)
            # marker frees head-0 attention chunks 2 groups early, pulling
            # the first A2A trigger ahead of the attention tail
            def p1_qk(b, qc, h, xt_c, ql):
                for (w_sb, b_sb, dst) in ((wq_sb, bq_sb, qt_sb),
                                          (wk_sb, bk_sb, kt_sb)):
                    pp = ps.tile([128, QC], F32, tag="A",
                                 name=f"psp{b}_{qc}_{h}")
                    for mb in range(n_mb):
                        nc.tensor.matmul(pp, w_sb[:, h, mb, :],
                                         xt_c[:, mb, :],
                                         start=(mb == 0),
                                         stop=(mb == n_mb - 1))
                    nc.vector.tensor_scalar_add(
                        dst[:, b, h, ql], pp, b_sb[:, h:h + 1])
                    yield n_mb * QC

            def p1_gen():
                deferred = []
                # b-alternating unit order spreads attention-chunk
                # availability (and so the attention phase's vector/scalar
                # work) evenly across the whole projection phase
                for qc in range(n_qc):
                    for b in range(batch):
                        ql = slice(qc * QC, (qc + 1) * QC)
                        if qc == 0 and b == 0:
                            xt_c = xt00  # loaded with the consts above
                        else:
                            xt_c = xin.tile([128, n_mb, QC], BF16, tag="xt",
                                            name=f"xt{b}_{qc}")
                            nc.sync.dma_start(out=xt_c, in_=xt_d[:, b, qc])
                        yield from p1_qk(b, qc, 0, xt_c, ql)
                        for pb4 in range(dpb):
                            pb = qc * dpb + pb4
                            pp = ps.tile([128, hpc * 128], F32, tag="A",
                                         name=f"psv{b}_{pb}")
                            for mb in range(n_mb):
                                nc.tensor.matmul(
                                    pp,
                                    xt_c[:, mb, pb4 * 128:(pb4 + 1) * 128],
                                    wv_sb[:, mb, :],
                                    start=(mb == 0), stop=(mb == n_mb - 1))
                            nc.vector.tensor_add(
                                v_sb[:, b, pb, :, :],
                                pp.rearrange("p (h d) -> p h d", h=hpc),
                                vb_sb.rearrange("p (h d) -> p h d", h=hpc))
                            yield n_mb * hpc * 128
                        markers.add(("v", b, qc))
                        if qc == n_qc - 1:
                            # defer the final units' h1 Q/K groups: the qc3
                            # head-0 attention chunks (which gate the first
                            # A2A trigger) unblock ~16us earlier, and these
                            # groups become extra post-trigger PE runway
                            deferred.append((b, qc, xt_c, ql))
                        else:
                            for h in range(1, hpc):
                                yield from p1_qk(b, qc, h, xt_c, ql)
                            markers.add(("full", b, qc))
                for b, qc, xt_c, ql in deferred:
                    for h in range(1, hpc):
                        yield from p1_qk(b, qc, h, xt_c, ql)
                    markers.add(("full", b, qc))


            # ---- stream 2: attention chunks, self-pipelined over k-tiles ----
            # qc-major order tracks unit availability; all h1 chunks of
            # qc1..3 are deferred behind the h0 chunks so the first A2A
            # trigger has ~43us of attention PE work after it — enough to
            # cover its rendezvous skew + transfer AND the CC-engine
            # serialization before the second A2A
            chunks = [(0, b, qc) for qc in range(n_qc) for b in range(batch)]
            chunks += [(1, b, qc) for qc in range(n_qc) for b in range(batch)]
            LAG = 2
            pend = []
            chain = {}

            def emit_score(ci, h, b, qc, kb, panel):
                dg = kb - qc * dpb
                off = 128 * dg if dg > 0 else 0
                np_ = QC - off
                st = ps.tile([128, QC], F32, tag="st", bufs=3,
                             name=f"st{ci}_{kb}")
                nc.tensor.matmul(st[:, :np_],
                                 kt_sb[:, b, h, kb * 128:(kb + 1) * 128],
                                 qt_sb[:, b, h, qc * QC + off:(qc + 1) * QC],
                                 start=True, stop=True)
                last_exp[0] = nc.scalar.activation(panel[:, kb, off:],
                                                   st[:, :np_], Exp)
                if dg >= 0:
                    band = slice(off, off + 128)
                    nc.vector.tensor_mul(panel[:, kb, band],
                                         panel[:, kb, band],
                                         mk_sb[:, dg, band])
                return np_

            def emit_av(ci, h, b, qc, kb, n_b, panel, zp, acc):
                dg = kb - qc * dpb
                off = 128 * dg if dg > 0 else 0
                nc.tensor.matmul(zp[:, off:], v_sb[:, b, kb, h, :],
                                 panel[:, kb, off:],
                                 start=(kb == 0), stop=(kb == n_b - 1),
                                 skip_group_check=True)
                # denominator partials: acc = sum_kb exp panel (masked rows
                # of diag tiles contribute only their valid [off:] columns).
                # bf16 accumulation: ~0.1% rms rounding per add, well inside
                # the error budget, and 16-bit DVE ops run ~1.7x faster
                if kb == 0:
                    nc.vector.tensor_copy(acc, panel[:, 0, :])
                else:
                    nc.vector.tensor_add(acc[:, off:], acc[:, off:],
                                         panel[:, kb, off:])
                return QC - off

            def emit_ship(ci):
                # reduce+broadcast the denominators with one all-ones matmul,
                # then normalize the full z psum into the A2A staging tile.
                # dps alternates psum tags so tail ship chains don't convoy
                # on a 2-bank rotation
                h, b, qc, zp, acc = chain.pop(ci)
                tg, bf = (("A", 2), ("st", 3))[ci % 2]
                dps = ps.tile([128, QC], F32, tag=tg, bufs=bf,
                              name=f"dps{ci}")
                nc.tensor.matmul(dps, ones_sb, acc, start=True, stop=True)
                rb = work.tile([128, QC], F32, tag="rb", bufs=2, name=f"rb{ci}")
                nc.vector.reciprocal_approx_fast(out=rb, in_=dps)
                zsb = work.tile([128, QC], BF16, tag="zsb", bufs=2,
                                name=f"zsb{ci}")
                nc.vector.tensor_mul(zsb, zp, rb)
                shard = b * n_qc + qc
                if h == 0:
                    nc.sync.dma_start(out=a2a_in[0][shard], in_=zsb)
                else:
                    nc.sync.dma_start(out=a2a_in1[0][shard], in_=zsb[:, 0:HQ])
                    nc.sync.dma_start(out=a2a_in1[1][shard], in_=zsb[:, HQ:])
                ship_n[h] += 1
                if ship_n[h] == batch * n_qc:
                    emit_cc(h)
                return QC

            def attn_gen():
                for ci, (h, b, qc) in enumerate(chunks):
                    gate = ("v", b, qc) if h == 0 else ("full", b, qc)
                    while gate not in markers:
                        yield ("blocked", gate)
                    n_b = (qc + 1) * dpb
                    panel = work.tile([128, n_kb, QC], BF16, tag="pt", bufs=2,
                                      name=f"pt{ci}")
                    zp = ps.tile([128, QC], F32, tag="z", bufs=3,
                                 name=f"zp{ci}")
                    acc = work.tile([128, QC], BF16, tag="acc", bufs=2,
                                    name=f"acc{ci}")
                    for k in range(n_b + LAG):
                        r = 0
                        if k < n_b:
                            r += emit_score(ci, h, b, qc, k, panel)
                        if 0 <= k - LAG < n_b:
                            r += emit_av(ci, h, b, qc, k - LAG, n_b, panel,
                                         zp, acc)
                        if k == 3 and pend:
                            r += emit_ship(pend.pop(0))
                        yield r
                    chain[ci] = (h, b, qc, zp, acc)
                    pend.append(ci)
                while pend:
                    yield emit_ship(pend.pop(0))
                # W_O chunks 0/1 after the last ship on sync: any earlier
                # and their 4MB of transfers sit AHEAD of the h1 ships in
                # the ring FIFO, delaying the second A2A trigger ~15us
                # (measured). The dedicated static avoids the xt-slot WAR.
                for mc in range(min(2, n_mc)):
                    nc.sync.dma_start(out=wo01_sb[:, mc], in_=wo_d[mc])
                    wo_tiles[mc] = wo01_sb[:, mc]
                # first A2A's zf tiles lead the scalar ring's post-attention
                # work (the gpsimd software-DGE ring would stall the second
                # A2A trigger behind their transfers)
                load_zf(0, (nc.scalar,))

            # ---- emission driver: fraction-paced interleave ----
            def drive(specs):
                gens = [g for g, _ in specs]
                totals = [float(t) for _, t in specs]
                spent = [0.0] * len(specs)
                alive = [True] * len(specs)
                blocked = [None] * len(specs)
                while any(alive):
                    cands = [i for i in range(len(specs)) if alive[i] and
                             (blocked[i] is None or blocked[i] in markers)]
                    assert cands, "emission driver deadlock"
                    i = min(cands, key=lambda j: spent[j] / totals[j])
                    blocked[i] = None
                    try:
                        item = next(gens[i])
                    except StopIteration:
                        alive[i] = False
                        continue
                    if isinstance(item, tuple):
                        blocked[i] = item[1]
                    else:
                        spent[i] += item

            p1_rows = batch * n_qc * (2 * hpc * n_mb * QC
                                      + dpb * n_mb * hpc * 128)
            at_rows = 0
            for (h, b, qc) in chunks:
                n_b = (qc + 1) * dpb
                for kb in range(n_b):
                    dg = kb - qc * dpb
                    off = 128 * dg if dg > 0 else 0
                    at_rows += 2 * (QC - off)
                at_rows += QC
            drive([(p1_gen(), p1_rows), (attn_gen(), at_rows)])

            # scalar-queue order at attention end: W_O chunks 2/3 into the
            # released qt/v slots (no collective wait; pass O needs them
            # mid-pass), then zf odds (which wait on cc1). Pinned after the
            # exps so the scheduler cannot hoist the waits into attention.
            for mc, tg in zip(range(2, n_mc), ("qt_sb", "v_sb")):
                t = stat.tile([128, n_heads, MC], BF16, tag=tg, name=f"wo{mc}")
                d = nc.scalar.dma_start(out=t, in_=wo_d[mc])
                if last_exp[0] is not None:
                    add_dep_helper(d.ins, last_exp[0].ins,
                                   reason="wo load after attention exps")
                wo_tiles[mc] = t
            # zf h1 loads, first-position-half first so pass O's pb 0/1
            # tiles unblock on cc1a alone. Three rings: gpsimd's triggers
            # precede these by construction; post-attention the sync queue
            # only has the pass-O output stores left, and those start after
            # cc1a completes anyway, so a cc1 wait blocks nothing.
            zf1_engs = (nc.gpsimd, nc.scalar, nc.sync)
            for p in range(2):
                for i in range(n_cores):
                    eng = zf1_engs[(p * n_cores + i) % 3]
                    d = eng.dma_start(out=zf_sb[:, 1, i, p * HQ:(p + 1) * HQ],
                                      in_=a2a_out1[p][i])
                    if eng is nc.scalar and last_exp[0] is not None:
                        add_dep_helper(d.ins, last_exp[0].ins,
                                       reason="zf load after attention exps")

            # ---- phase 3: output projection, two passes ----
            # Pass E runs the first-A2A heads for ALL output tiles staged to
            # SBUF in bf16; pass O adds the remaining heads once cc1's zf
            # tiles have arrived. (global head g = i*hpc + h for source
            # core i, local head h; zf is [128, h, i, rows])
            late_hi = [(h, i) for h in range(1, hpc) for i in range(n_cores)]
            # pass-E staging reuses kt's slot (dead after the last score MM)
            osbe = stat.tile([128, n_mc * n_pb, MC], BF16, tag="kt_sb")
            # pb-major: pass O's first tiles only need the first A2A1 half
            tiles3 = [(mc, pb) for pb in range(n_pb) for mc in range(n_mc)]
            for n, (mc, pb) in enumerate(tiles3):
                tg, bf = (("A", 2), ("st", 3))[n % 2]
                pp = ps.tile([128, MC], F32, tag=tg, bufs=bf,
                             name=f"pse{mc}_{pb}")
                for i in range(n_cores):
                    nc.tensor.matmul(
                        pp, zf_sb[:, 0, i, pb * 128:(pb + 1) * 128],
                        wo_tiles[mc][:, i * hpc, :],
                        start=(i == 0), stop=(i == n_cores - 1))
                nc.vector.tensor_copy(osbe[:, n, :], pp)
            for n, (mc, pb) in enumerate(tiles3):
                tg, bf = (("A", 2), ("st", 3))[n % 2]
                pp = ps.tile([128, MC], F32, tag=tg, bufs=bf,
                             name=f"pso{mc}_{pb}")
                for j, (h, i) in enumerate(late_hi):
                    nc.tensor.matmul(
                        pp, zf_sb[:, h, i, pb * 128:(pb + 1) * 128],
                        wo_tiles[mc][:, i * hpc + h, :],
                        start=(j == 0), stop=(j == len(late_hi) - 1))
                osb = work.tile([128, MC], BF16, tag="osb", bufs=2,
                                name=f"osb{mc}_{pb}")
                nc.vector.tensor_add(osb, pp, osbe[:, n, :])
                # alternate output rings so the final flush isn't serialized
                oeng = nc.sync if n % 2 == 0 else nc.scalar
                oeng.dma_start(out=out_d[pb, mc], in_=osb)

    nc.compile()
    return nc


def make_in_maps(inputs, cfg=FULL):
    c = _derived(cfg)
    hpc, QC = c["hpc"], c["qc_size"]
    n_mb, n_dg = c["n_mb"], c["n_dg"]
    d_model, seq, batch = c["d_model"], c["seq"], c["batch"]
    residual = np.asarray(inputs["residual"], np.float32)
    W_Q = np.asarray(inputs["W_Q"], np.float32)
    W_K = np.asarray(inputs["W_K"], np.float32)
    W_V = np.asarray(inputs["W_V"], np.float32)
    W_O = np.asarray(inputs["W_O"], np.float32)
    b_Q = np.asarray(inputs["b_Q"], np.float32)
    b_K = np.asarray(inputs["b_K"], np.float32)
    b_V = np.asarray(inputs["b_V"], np.float32)
    scale = 1.0 / ATTN_SCALE

    # X^T packed per q-chunk [128, batch, n_qc, n_mb, QC]:
    # [p, b, qc, mb, s'] = residual[b, qc*QC+s', mb*128+p]
    n_qc = c["n_qc"]
    xt = np.ascontiguousarray(
        residual.reshape(batch, n_qc, QC, n_mb, 128).transpose(4, 0, 1, 3, 2)
    ).astype(NP_BF16)
    # W_O packed chunk-major [n_mc, 128, n_heads, 512]:
    # [mc, p, g, m'] = W_O[g, p, mc*512+m']
    n_mc = c["n_mc"]
    wo = np.ascontiguousarray(
        W_O.transpose(1, 0, 2).reshape(128, c["n_heads"], n_mc, 512)
        .transpose(2, 0, 1, 3)).astype(NP_BF16)
    # causal {0,1} masks packed [128, n_dg, QC]
    masks = np.zeros((128, n_dg, QC), np.float32)
    pk = np.arange(128)[:, None]
    fq = np.arange(QC)[None, :]
    for dg in range(n_dg):
        masks[:, dg, :] = (fq >= pk + 128 * dg).astype(np.float32)
    masks = masks.astype(NP_BF16)

    in_maps = []
    for core in range(c["n_cores"]):
        hs = slice(core * hpc, (core + 1) * hpc)
        # [128, hpc, n_mb, 128]: [p, h, mb, d] = W[h, mb*128+p, d]
        wq = np.ascontiguousarray(
            (W_Q[hs] * scale).reshape(hpc, n_mb, 128, 128).transpose(2, 0, 1, 3)
        ).astype(NP_BF16)
        wk = np.ascontiguousarray(
            W_K[hs].reshape(hpc, n_mb, 128, 128).transpose(2, 0, 1, 3)
        ).astype(NP_BF16)
        # [128, n_mb, hpc*128]: [p, mb, (h d)] = W_V[h, mb*128+p, d]
        wv = np.ascontiguousarray(
            W_V[hs].reshape(hpc, n_mb, 128, 128).transpose(2, 1, 0, 3)
            .reshape(128, n_mb, hpc * 128)).astype(NP_BF16)
        bq = np.ascontiguousarray((b_Q[hs] * scale).T).astype(np.float32)
        bk = np.ascontiguousarray(b_K[hs].T).astype(np.float32)
        bv = np.ascontiguousarray(b_V[hs].reshape(hpc * 128)).astype(np.float32)
        in_maps.append({
            "xt": xt, "wq": wq, "wk": wk, "wv": wv, "wo": wo,
            "bq": bq, "bk": bk, "bv": bv, "mk": masks,
        })
    return in_maps


def assemble_output(inputs, shards, cfg=FULL):
    c = _derived(cfg)
    residual = np.asarray(inputs["residual"], np.float32)
    b_O = np.asarray(inputs["b_O"], np.float32)
    # each shard is tile-major [n_pb, n_mc, 128, 512] -> [rows, d_model]
    rows, d_model = c["rows"], c["d_model"]
    flat = [np.asarray(s).astype(np.float32).transpose(0, 2, 1, 3)
            .reshape(rows, d_model) for s in shards]
    out = np.concatenate(flat, axis=0)
    out = out.reshape(c["batch"], c["seq"], c["d_model"]) + b_O
    return residual, out.astype(np.float32)


_NC_CACHE = {}


def _get_nc():
    if "nc" not in _NC_CACHE:
        _NC_CACHE["nc"] = build_graph(FULL)
    return _NC_CACHE["nc"]


def run(inputs, trace=False):
    nc = _get_nc()
    in_maps = make_in_maps(inputs, FULL)
    try:
        res = run_bass_kernel_spmd(nc, in_maps, list(range(FULL["n_cores"])),
                                   trace=trace)
    except Exception:
        # a previous bad run can leave the remote device wedged for one
        # attempt; give it a moment and retry once
        import time
        time.sleep(60)
        res = run_bass_kernel_spmd(nc, in_maps, list(range(FULL["n_cores"])),
                                   trace=trace)
    shards = [res.results[i]["out"] for i in range(FULL["n_cores"])]
    residual, out = assemble_output(inputs, shards, FULL)
    return (residual, out), res


def kernel(**inputs):
    (residual, out), _ = run(inputs, trace=False)
    return (residual, out)



# revision 38
# speedup vs baseline: 1.0359x; 1.0359x over previous
"""Distributed multi-head causal attention for 8 TRN2 NeuronCores.

Problem: residual [2, 2048, 2048] f32 -> (residual, attn_out [2, 2048, 2048])
  q/k/v = residual @ W_{Q,K,V} + b  (16 heads, d_head 128)
  scores = q k^T / sqrt(128), causal mask, softmax
  out = (pattern @ v) @ W_O + b_O

Sharding: tensor-parallel over heads. Core c computes QKV projections and
attention for heads 2c, 2c+1 over both batches, producing z^T (the
pre-output-projection activations). Two 8-core AllToAlls (one per local
head) redistribute z^T from head-sharded to position-sharded: shard j covers
positions [512*j, 512*(j+1)) of the flattened [batch*seq] axis. After the
A2A each core holds all 16 heads for its own 512 positions and computes the
output projection for just those rows. The host concatenates the 8 shards.

All matmuls in bf16 (inputs pre-cast and pre-packed on host so every load is
one large contiguous DMA), accumulation f32 in PSUM:
  Q^T/K^T [dh, pos] = W^T X^T    (lhsT = W [model, dh], rhs = X^T)
  V [pos, dh*hpc]   = X W_V      (lhsT = X^T tile,      rhs = W_V heads)
  S^T [k, q]        = K Q^T      (lhsT = K^T tile,      rhs = Q^T)
  z^T [dh, q]       = V^T P^T    (lhsT = V tile,        rhs = P^T = exp(S^T))
  den [*, q]        = J acc      (lhsT = all-ones,      rhs = sum_k P^T)
  out [pos, m]      = z W_O      (lhsT = z^T tile,      rhs = W_O)

The emission driver software-pipelines the phases on the single PE queue:
attention chunks (h, b, qc) become runnable as soon as phase 1 has produced
q/k/v for (b, <=qc) (causality: chunk qc only attends keys < (qc+1)*512), so
attention matmuls interleave with later projection units and the attention
phase's scalar-engine exps and vector-engine denominator accumulations hide
under projection PE work. The AV pass is one full-width [128, 512] matmul
per k-tile; softmax denominators come from vector adds of the exp panels
(acc = sum_k P^T) reduced+broadcast by a single all-ones matmul per chunk.
Pass E of the output projection (first-A2A heads) starts the moment the last
attention matmul retires; pass O adds the second-A2A heads.

Schedule refinements over the first working version: output stores are
tile-major contiguous 128KB writes (host reassembles); W_O chunks 0/1 load
into a dedicated static (no xt-slot WAR); zf h1 loads spread over all three
DMA rings; pass-E staging aliases kt's slot and W_O chunk 3 aliases v's
slot; the PE warmup extends through the DMA-bound start (16x512 + 24x256
dummies) so the p-state never drops before the first real matmul.

Measured no-gos (do not retry): fp8 double-pump matmuls (any fp8 operand in
the chain gives ~3.5%+ output error vs the 2e-2 budget; z is itself an
average of random v's so quantization noise does not wash out); splitting
weight loads below 4KB-per-partition segments (ring BW drops ~2x); bulk
transfers on the gpsimd software-DGE ring; W_O loads queued ahead of the h1
ships in the sync ring FIFO (delays the second A2A trigger ~15us); zf h0
loads on gpsimd between the A2A triggers (cc0's rendezvous wait can outlast
the h1 ships under peer skew and then delays cc1); splitting the second A2A
into position-halves (halves the ship DMA segment size, slower overall);
PSUM st=4/z=2 rebalance (zp rotation convoys the attention chunks).
"""

import numpy as np
import ml_dtypes

import concourse.bass as bass
import concourse.tile as tile
from concourse import bacc, mybir
from concourse.bass_utils import run_bass_kernel_spmd
from concourse.tile_rust import add_dep_helper

BF16 = mybir.dt.bfloat16
F32 = mybir.dt.float32
NP_BF16 = ml_dtypes.bfloat16

FULL = dict(n_heads=16, d_model=2048, d_head=128, batch=2, seq=2048, n_cores=8)
ATTN_SCALE = float(np.sqrt(128.0))


def _derived(cfg):
    d = dict(cfg)
    d["hpc"] = d["n_heads"] // d["n_cores"]             # heads per core
    d["rows"] = d["batch"] * d["seq"] // d["n_cores"]   # out rows per core
    d["qc_size"] = d["rows"]                            # q-chunk == A2A shard
    assert d["qc_size"] <= 512
    d["n_qc"] = d["seq"] // d["qc_size"]                # q chunks per batch
    d["n_kb"] = d["seq"] // 128                         # k blocks per batch
    d["n_mb"] = d["d_model"] // 128                     # model-dim blocks
    d["n_dg"] = d["qc_size"] // 128                     # diag offsets per chunk
    d["n_mc"] = d["d_model"] // 512                     # out m-chunks
    d["n_pb"] = d["rows"] // 128                        # out pos-blocks
    assert d["n_qc"] * d["batch"] == d["n_cores"]
    assert d["d_head"] == 128
    return d


def build_graph(cfg=FULL, enable_asserts=False):
    c = _derived(cfg)
    hpc, QC = c["hpc"], c["qc_size"]
    n_qc, n_kb, n_mb, n_dg = c["n_qc"], c["n_kb"], c["n_mb"], c["n_dg"]
    n_mc, n_pb, rows = c["n_mc"], c["n_pb"], c["rows"]
    n_heads, d_model, seq = c["n_heads"], c["d_model"], c["seq"]
    batch, n_cores = c["batch"], c["n_cores"]
    dpb = QC // 128
    MC = 512

    nc = bacc.Bacc("TRN2", target_bir_lowering=False, debug=False,
                   enable_asserts=enable_asserts, num_devices=n_cores)

    # all inputs pre-packed on host into [128, ...] partition-major layouts
    xt_d = nc.dram_tensor("xt", [128, batch, n_qc, n_mb, QC], BF16,
                          kind="ExternalInput")
    wq_d = nc.dram_tensor("wq", [128, hpc, n_mb, 128], BF16, kind="ExternalInput")
    wk_d = nc.dram_tensor("wk", [128, hpc, n_mb, 128], BF16, kind="ExternalInput")
    wv_d = nc.dram_tensor("wv", [128, n_mb, hpc * 128], BF16, kind="ExternalInput")
    # chunk-major so each 2MB W_O chunk load is fully contiguous
    wo_d = nc.dram_tensor("wo", [n_mc, 128, n_heads, MC], BF16,
                          kind="ExternalInput")
    bq_d = nc.dram_tensor("bq", [128, hpc], F32, kind="ExternalInput")
    bk_d = nc.dram_tensor("bk", [128, hpc], F32, kind="ExternalInput")
    bv_d = nc.dram_tensor("bv", [hpc * 128], F32, kind="ExternalInput")
    mk_d = nc.dram_tensor("mk", [128, n_dg, QC], BF16, kind="ExternalInput")
    # bf16 output halves the final DMA tail; the host adds b_O in f32.
    # tile-major layout so every output store is one contiguous 128KB write
    # (row-major [rows, d_model] made the final DMAs 1KB-segment strided and
    # the kernel-end barrier waited ~10us for them to drain)
    out_d = nc.dram_tensor("out", [n_pb, n_mc, 128, MC], BF16,
                           kind="ExternalOutput")

    rg = [list(range(n_cores))]
    Exp = mybir.ActivationFunctionType.Exp

    with tile.TileContext(nc) as tc:
        with (
            tc.tile_pool(name="stat", bufs=1) as stat,
            tc.tile_pool(name="xin", bufs=2) as xin,
            tc.tile_pool(name="work", bufs=3) as work,
            tc.tile_pool(name="ps", bufs=2, space="PSUM") as ps,
            tc.tile_pool(name="dram", bufs=1, space="DRAM") as dram,
        ):
            wq_sb = stat.tile([128, hpc, n_mb, 128], BF16)
            wk_sb = stat.tile([128, hpc, n_mb, 128], BF16)
            wv_sb = stat.tile([128, n_mb, hpc * 128], BF16, tag="wvzf")
            qt_sb = stat.tile([128, batch, hpc, seq], BF16)
            kt_sb = stat.tile([128, batch, hpc, seq], BF16)
            v_sb = stat.tile([128, batch, n_kb, hpc, 128], BF16)
            # W_O chunks 0/1 get a dedicated static so their 2MB loads can
            # ride the sync ring mid-phase-1 (it idles once the xt stream
            # ends) instead of queueing behind the last attention ships
            wo01_sb = stat.tile([128, 2, n_heads, MC], BF16)
            bq_sb = stat.tile([128, hpc], F32)
            bk_sb = stat.tile([128, hpc], F32)
            vb_sb = stat.tile([128, hpc * 128], F32)
            mk_sb = stat.tile([128, n_dg, QC], BF16)
            ones_sb = stat.tile([128, 128], BF16)
            # h-major so each A2A's zf tiles land with ONE strided DMA
            zf_sb = stat.tile([128, hpc, n_cores, rows], BF16, tag="wvzf")

            a2a_in = [dram.tile([n_cores, 128, rows], BF16, name=f"a2ai{h}")
                      for h in range(hpc)]
            a2a_out = [dram.tile([n_cores, 128, rows], BF16, name=f"a2ao{h}")
                       for h in range(hpc)]

            # warmup memsets lead the vector queue so the dummy matmuls can
            # start the p-state ramp immediately after the engine barrier
            grb_sb = stat.tile([128, QC], BF16)
            nc.vector.memset(ones_sb, 1.0)
            nc.vector.memset(grb_sb[0:1, 0:1], 1.0)

            # startup loads spread over the three DMA-capable rings
            # (scalar/sync/gpsimd) in exact first-consumption order; wq h0
            # and wv split in halves so the first chains start as soon as
            # their leading mb blocks land instead of waiting for whole
            # weight tensors
            # big weights all on the scalar HWDGE queue as single descriptors
            # (issue cost ~0.6us each), ordered to match first consumption
            # (Q h0, K h0, V, Q h1, K h1); small consts on gpsimd; xt chunks
            # stream on sync alone so chunk N+1 is never stuck behind a
            # weight load. NOTE: splitting the weight loads into halves was
            # tried (v4) and is SLOWER — it halves the strided DMA segment
            # size (4KB -> 2KB) and ring bandwidth drops more than the
            # earlier start gains.
            xt00 = xin.tile([128, n_mb, QC], BF16, tag="xt", name="xt0_0")
            nc.scalar.dma_start(out=wq_sb[:, 0], in_=wq_d[:, 0])
            nc.sync.dma_start(out=xt00[:, 0:4], in_=xt_d[:, 0, 0, 0:4])
            nc.scalar.dma_start(out=xt00[:, 4:8], in_=xt_d[:, 0, 0, 4:8])
            nc.sync.dma_start(out=xt00[:, 8:12], in_=xt_d[:, 0, 0, 8:12])
            nc.scalar.dma_start(out=wv_sb, in_=wv_d[:])
            nc.sync.dma_start(out=xt00[:, 12:16], in_=xt_d[:, 0, 0, 12:16])
            for h in range(1, hpc):
                nc.scalar.dma_start(out=wq_sb[:, h], in_=wq_d[:, h])
            nc.gpsimd.dma_start(out=bq_sb, in_=bq_d[:])
            nc.gpsimd.dma_start(out=bk_sb, in_=bk_d[:])
            # K weights ride the otherwise-idle gpsimd ring so the first K
            # chains never queue behind the Q/V loads on scalar
            nc.gpsimd.dma_start(out=wk_sb, in_=wk_d[:])
            bv_ap = bv_d.ap()
            nc.gpsimd.dma_start(
                out=vb_sb,
                in_=bass.AP(tensor=bv_ap.tensor, offset=bv_ap.offset,
                            ap=[[0, 128]] + list(bv_ap.ap)))
            nc.gpsimd.dma_start(out=mk_sb, in_=mk_d[:])

            # PE warmup: full-width dummy matmuls bridge the gap between
            # the engine barrier and the first weight arrival so the p-state
            # ramp (needs ~3us of continuous HIGH-utilization busy to reach
            # max clock) completes on dummy work. The tail of the warmup is
            # 256-row dummies: fine-grained so a late-arriving first weight
            # is delayed at most ~130ns, but enough of them to keep the
            # clock ramped through the ~+21us DMA-bound start.
            for w in range(16):
                dmy = ps.tile([128, QC], F32, tag="A", name=f"wrm{w}")
                nc.tensor.matmul(dmy, ones_sb, grb_sb,
                                 start=True, stop=True)
            for w in range(24):
                dmy = ps.tile([128, QC // 2], F32, tag="A",
                              name=f"wrs{w}")
                nc.tensor.matmul(dmy, ones_sb, grb_sb[:, 0:QC // 2],
                                 start=True, stop=True)

            wo_tiles = {}
            last_exp = [None]
            markers = set()
            ship_n = [0] * hpc

            def emit_cc(h):
                nc.gpsimd.collective_compute(
                    "AllToAll", mybir.AluOpType.bypass, replica_groups=rg,
                    ins=[a2a_in[h].opt()], outs=[a2a_out[h].opt()])


            def load_zf(h, engs):
                # 8 fully-contiguous 128KB loads (a strided single-descriptor
                # variant measured 55GB/s vs ~350 contiguous), spread across
                # the given rings
                for i in range(n_cores):
                    eng = engs[i % len(engs)]
                    d = eng.dma_start(out=zf_sb[:, h, i, :],
                                      in_=a2a_out[h][i])
                    if eng is nc.scalar and last_exp[0] is not None:
                        # scalar ring also runs the exps: pin after the final
                        # exp so the scheduler cannot hoist the collective-
                        # completion wait into the middle of the attention
                        add_dep_helper(d.ins, last_exp[0].ins,
                                       reason="zf load after attention exps")

            # ---- stream 1: phase-1 QKV projection units ----
            # group order [Q h0, K h0, V, Q h1, K h1]: the ("v", b, qc)
            # marker frees head-0 attention chunks 2 groups early, pulling
            # the first A2A trigger ahead of the attention tail
            def p1_qk(b, qc, h, xt_c, ql):
                for (w_sb, b_sb, dst) in ((wq_sb, bq_sb, qt_sb),
                                          (wk_sb, bk_sb, kt_sb)):
                    pp = ps.tile([128, QC], F32, tag="A",
                                 name=f"psp{b}_{qc}_{h}")
                    for mb in range(n_mb):
                        nc.tensor.matmul(pp, w_sb[:, h, mb, :],
                                         xt_c[:, mb, :],
                                         start=(mb == 0),
                                         stop=(mb == n_mb - 1))
                    nc.vector.tensor_scalar_add(
                        dst[:, b, h, ql], pp, b_sb[:, h:h + 1])
                    yield n_mb * QC

            def p1_gen():
                deferred = []
                # b-alternating unit order spreads attention-chunk
                # availability (and so the attention phase's vector/scalar
                # work) evenly across the whole projection phase
                for qc in range(n_qc):
                    for b in range(batch):
                        ql = slice(qc * QC, (qc + 1) * QC)
                        if qc == 0 and b == 0:
                            xt_c = xt00  # loaded with the consts above
                        else:
                            xt_c = xin.tile([128, n_mb, QC], BF16, tag="xt",
                                            name=f"xt{b}_{qc}")
                            nc.sync.dma_start(out=xt_c, in_=xt_d[:, b, qc])
                        yield from p1_qk(b, qc, 0, xt_c, ql)
                        for pb4 in range(dpb):
                            pb = qc * dpb + pb4
                            pp = ps.tile([128, hpc * 128], F32, tag="A",
                                         name=f"psv{b}_{pb}")
                            for mb in range(n_mb):
                                nc.tensor.matmul(
                                    pp,
                                    xt_c[:, mb, pb4 * 128:(pb4 + 1) * 128],
                                    wv_sb[:, mb, :],
                                    start=(mb == 0), stop=(mb == n_mb - 1))
                            nc.vector.tensor_add(
                                v_sb[:, b, pb, :, :],
                                pp.rearrange("p (h d) -> p h d", h=hpc),
                                vb_sb.rearrange("p (h d) -> p h d", h=hpc))
                            yield n_mb * hpc * 128
                        markers.add(("v", b, qc))
                        if qc == n_qc - 1:
                            # defer the final units' h1 Q/K groups: the qc3
                            # head-0 attention chunks (which gate the first
                            # A2A trigger) unblock ~16us earlier, and these
                            # groups become extra post-trigger PE runway
                            deferred.append((b, qc, xt_c, ql))
                        else:
                            for h in range(1, hpc):
                                yield from p1_qk(b, qc, h, xt_c, ql)
                            markers.add(("full", b, qc))
                for b, qc, xt_c, ql in deferred:
                    for h in range(1, hpc):
                        yield from p1_qk(b, qc, h, xt_c, ql)
                    markers.add(("full", b, qc))


            # ---- stream 2: attention chunks, self-pipelined over k-tiles ----
            # qc-major order tracks unit availability; all h1 chunks of
            # qc1..3 are deferred behind the h0 chunks so the first A2A
            # trigger has ~43us of attention PE work after it — enough to
            # cover its rendezvous skew + transfer AND the CC-engine
            # serialization before the second A2A
            chunks = [(0, b, qc) for qc in range(n_qc) for b in range(batch)]
            chunks += [(1, b, qc) for qc in range(n_qc) for b in range(batch)]
            LAG = 2
            pend = []
            chain = {}

            def emit_score(ci, h, b, qc, kb, panel):
                dg = kb - qc * dpb
                off = 128 * dg if dg > 0 else 0
                np_ = QC - off
                st = ps.tile([128, QC], F32, tag="st", bufs=3,
                             name=f"st{ci}_{kb}")
                nc.tensor.matmul(st[:, :np_],
                                 kt_sb[:, b, h, kb * 128:(kb + 1) * 128],
                                 qt_sb[:, b, h, qc * QC + off:(qc + 1) * QC],
                                 start=True, stop=True)
                last_exp[0] = nc.scalar.activation(panel[:, kb, off:],
                                                   st[:, :np_], Exp)
                if dg >= 0:
                    band = slice(off, off + 128)
                    nc.vector.tensor_mul(panel[:, kb, band],
                                         panel[:, kb, band],
                                         mk_sb[:, dg, band])
                return np_

            def emit_av(ci, h, b, qc, kb, n_b, panel, zp, acc):
                dg = kb - qc * dpb
                off = 128 * dg if dg > 0 else 0
                nc.tensor.matmul(zp[:, off:], v_sb[:, b, kb, h, :],
                                 panel[:, kb, off:],
                                 start=(kb == 0), stop=(kb == n_b - 1),
                                 skip_group_check=True)
                # denominator partials: acc = sum_kb exp panel (masked rows
                # of diag tiles contribute only their valid [off:] columns).
                # bf16 accumulation: ~0.1% rms rounding per add, well inside
                # the error budget, and 16-bit DVE ops run ~1.7x faster
                if kb == 0:
                    nc.vector.tensor_copy(acc, panel[:, 0, :])
                else:
                    nc.vector.tensor_add(acc[:, off:], acc[:, off:],
                                         panel[:, kb, off:])
                return QC - off

            def emit_ship(ci):
                # reduce+broadcast the denominators with one all-ones matmul,
                # then normalize the full z psum into the A2A staging tile.
                # dps alternates psum tags so tail ship chains don't convoy
                # on a 2-bank rotation
                h, b, qc, zp, acc = chain.pop(ci)
                tg, bf = (("A", 2), ("st", 3))[ci % 2]
                dps = ps.tile([128, QC], F32, tag=tg, bufs=bf,
                              name=f"dps{ci}")
                nc.tensor.matmul(dps, ones_sb, acc, start=True, stop=True)
                rb = work.tile([128, QC], F32, tag="rb", bufs=2, name=f"rb{ci}")
                nc.vector.reciprocal_approx_fast(out=rb, in_=dps)
                zsb = work.tile([128, QC], BF16, tag="zsb", bufs=2,
                                name=f"zsb{ci}")
                nc.vector.tensor_mul(zsb, zp, rb)
                shard = b * n_qc + qc
                nc.sync.dma_start(out=a2a_in[h][shard], in_=zsb)
                ship_n[h] += 1
                if ship_n[h] == batch * n_qc:
                    emit_cc(h)
                return QC

            def attn_gen():
                for ci, (h, b, qc) in enumerate(chunks):
                    gate = ("v", b, qc) if h == 0 else ("full", b, qc)
                    while gate not in markers:
                        yield ("blocked", gate)
                    n_b = (qc + 1) * dpb
                    panel = work.tile([128, n_kb, QC], BF16, tag="pt", bufs=2,
                                      name=f"pt{ci}")
                    zp = ps.tile([128, QC], F32, tag="z", bufs=3,
                                 name=f"zp{ci}")
                    acc = work.tile([128, QC], BF16, tag="acc", bufs=2,
                                    name=f"acc{ci}")
                    for k in range(n_b + LAG):
                        r = 0
                        if k < n_b:
                            r += emit_score(ci, h, b, qc, k, panel)
                        if 0 <= k - LAG < n_b:
                            r += emit_av(ci, h, b, qc, k - LAG, n_b, panel,
                                         zp, acc)
                        if k == 3 and pend:
                            r += emit_ship(pend.pop(0))
                        yield r
                    chain[ci] = (h, b, qc, zp, acc)
                    pend.append(ci)
                while pend:
                    yield emit_ship(pend.pop(0))
                # W_O chunks 0/1 after the last ship on sync: any earlier
                # and their 4MB of transfers sit AHEAD of the h1 ships in
                # the ring FIFO, delaying the second A2A trigger ~15us
                # (measured). The dedicated static avoids the xt-slot WAR.
                for mc in range(min(2, n_mc)):
                    nc.sync.dma_start(out=wo01_sb[:, mc], in_=wo_d[mc])
                    wo_tiles[mc] = wo01_sb[:, mc]
                # first A2A's zf tiles lead the scalar ring's post-attention
                # work (the gpsimd software-DGE ring would stall the second
                # A2A trigger behind their transfers)
                load_zf(0, (nc.scalar,))

            # ---- emission driver: fraction-paced interleave ----
            def drive(specs):
                gens = [g for g, _ in specs]
                totals = [float(t) for _, t in specs]
                spent = [0.0] * len(specs)
                alive = [True] * len(specs)
                blocked = [None] * len(specs)
                while any(alive):
                    cands = [i for i in range(len(specs)) if alive[i] and
                             (blocked[i] is None or blocked[i] in markers)]
                    assert cands, "emission driver deadlock"
                    i = min(cands, key=lambda j: spent[j] / totals[j])
                    blocked[i] = None
                    try:
                        item = next(gens[i])
                    except StopIteration:
                        alive[i] = False
                        continue
                    if isinstance(item, tuple):
                        blocked[i] = item[1]
                    else:
                        spent[i] += item

            p1_rows = batch * n_qc * (2 * hpc * n_mb * QC
                                      + dpb * n_mb * hpc * 128)
            at_rows = 0
            for (h, b, qc) in chunks:
                n_b = (qc + 1) * dpb
                for kb in range(n_b):
                    dg = kb - qc * dpb
                    off = 128 * dg if dg > 0 else 0
                    at_rows += 2 * (QC - off)
                at_rows += QC
            drive([(p1_gen(), p1_rows), (attn_gen(), at_rows)])

            # scalar-queue order at attention end: W_O chunks 2/3 into the
            # released qt/v slots (no collective wait; pass O needs them
            # mid-pass), then zf odds (which wait on cc1). Pinned after the
            # exps so the scheduler cannot hoist the waits into attention.
            for mc, tg in zip(range(2, n_mc), ("qt_sb", "v_sb")):
                t = stat.tile([128, n_heads, MC], BF16, tag=tg, name=f"wo{mc}")
                d = nc.scalar.dma_start(out=t, in_=wo_d[mc])
                if last_exp[0] is not None:
                    add_dep_helper(d.ins, last_exp[0].ins,
                                   reason="wo load after attention exps")
                wo_tiles[mc] = t
            for h in range(1, hpc):
                # gpsimd ring is idle after the second A2A trigger, and the
                # trigger precedes these loads there by construction — so
                # the cc1-completion wait can't block anything else
                # three rings: post-attention the sync queue only has the
                # pass-O output stores left, and those start after cc1
                # completes anyway, so a cc1 wait on sync blocks nothing
                load_zf(h, (nc.gpsimd, nc.scalar, nc.sync))

            # ---- phase 3: output projection, two passes ----
            # Pass E runs the first-A2A heads for ALL output tiles staged to
            # SBUF in bf16; pass O adds the remaining heads once cc1's zf
            # tiles have arrived. (global head g = i*hpc + h for source
            # core i, local head h; zf is [128, h, i, rows])
            late_hi = [(h, i) for h in range(1, hpc) for i in range(n_cores)]
            # pass-E staging reuses kt's slot (dead after the last score MM)
            osbe = stat.tile([128, n_mc * n_pb, MC], BF16, tag="kt_sb")
            tiles3 = [(mc, pb) for mc in range(n_mc) for pb in range(n_pb)]
            for n, (mc, pb) in enumerate(tiles3):
                tg, bf = (("A", 2), ("st", 3))[n % 2]
                pp = ps.tile([128, MC], F32, tag=tg, bufs=bf,
                             name=f"pse{mc}_{pb}")
                for i in range(n_cores):
                    nc.tensor.matmul(
                        pp, zf_sb[:, 0, i, pb * 128:(pb + 1) * 128],
                        wo_tiles[mc][:, i * hpc, :],
                        start=(i == 0), stop=(i == n_cores - 1))
                nc.vector.tensor_copy(osbe[:, n, :], pp)
            for n, (mc, pb) in enumerate(tiles3):
                tg, bf = (("A", 2), ("st", 3))[n % 2]
                pp = ps.tile([128, MC], F32, tag=tg, bufs=bf,
                             name=f"pso{mc}_{pb}")
                for j, (h, i) in enumerate(late_hi):
                    nc.tensor.matmul(
                        pp, zf_sb[:, h, i, pb * 128:(pb + 1) * 128],
                        wo_tiles[mc][:, i * hpc + h, :],
                        start=(j == 0), stop=(j == len(late_hi) - 1))
                osb = work.tile([128, MC], BF16, tag="osb", bufs=2,
                                name=f"osb{mc}_{pb}")
                nc.vector.tensor_add(osb, pp, osbe[:, n, :])
                # alternate output rings so the final flush isn't serialized
                oeng = nc.sync if n % 2 == 0 else nc.scalar
                oeng.dma_start(out=out_d[pb, mc], in_=osb)

    nc.compile()
    return nc


def make_in_maps(inputs, cfg=FULL):
    c = _derived(cfg)
    hpc, QC = c["hpc"], c["qc_size"]
    n_mb, n_dg = c["n_mb"], c["n_dg"]
    d_model, seq, batch = c["d_model"], c["seq"], c["batch"]
    residual = np.asarray(inputs["residual"], np.float32)
    W_Q = np.asarray(inputs["W_Q"], np.float32)
    W_K = np.asarray(inputs["W_K"], np.float32)
    W_V = np.asarray(inputs["W_V"], np.float32)
    W_O = np.asarray(inputs["W_O"], np.float32)
    b_Q = np.asarray(inputs["b_Q"], np.float32)
    b_K = np.asarray(inputs["b_K"], np.float32)
    b_V = np.asarray(inputs["b_V"], np.float32)
    scale = 1.0 / ATTN_SCALE

    # X^T packed per q-chunk [128, batch, n_qc, n_mb, QC]:
    # [p, b, qc, mb, s'] = residual[b, qc*QC+s', mb*128+p]
    n_qc = c["n_qc"]
    xt = np.ascontiguousarray(
        residual.reshape(batch, n_qc, QC, n_mb, 128).transpose(4, 0, 1, 3, 2)
    ).astype(NP_BF16)
    # W_O packed chunk-major [n_mc, 128, n_heads, 512]:
    # [mc, p, g, m'] = W_O[g, p, mc*512+m']
    n_mc = c["n_mc"]
    wo = np.ascontiguousarray(
        W_O.transpose(1, 0, 2).reshape(128, c["n_heads"], n_mc, 512)
        .transpose(2, 0, 1, 3)).astype(NP_BF16)
    # causal {0,1} masks packed [128, n_dg, QC]
    masks = np.zeros((128, n_dg, QC), np.float32)
    pk = np.arange(128)[:, None]
    fq = np.arange(QC)[None, :]
    for dg in range(n_dg):
        masks[:, dg, :] = (fq >= pk + 128 * dg).astype(np.float32)
    masks = masks.astype(NP_BF16)

    in_maps = []
    for core in range(c["n_cores"]):
        hs = slice(core * hpc, (core + 1) * hpc)
        # [128, hpc, n_mb, 128]: [p, h, mb, d] = W[h, mb*128+p, d]
        wq = np.ascontiguousarray(
            (W_Q[hs] * scale).reshape(hpc, n_mb, 128, 128).transpose(2, 0, 1, 3)
        ).astype(NP_BF16)
        wk = np.ascontiguousarray(
            W_K[hs].reshape(hpc, n_mb, 128, 128).transpose(2, 0, 1, 3)
        ).astype(NP_BF16)
        # [128, n_mb, hpc*128]: [p, mb, (h d)] = W_V[h, mb*128+p, d]
        wv = np.ascontiguousarray(
            W_V[hs].reshape(hpc, n_mb, 128, 128).transpose(2, 1, 0, 3)
            .reshape(128, n_mb, hpc * 128)).astype(NP_BF16)
        bq = np.ascontiguousarray((b_Q[hs] * scale).T).astype(np.float32)
        bk = np.ascontiguousarray(b_K[hs].T).astype(np.float32)
        bv = np.ascontiguousarray(b_V[hs].reshape(hpc * 128)).astype(np.float32)
        in_maps.append({
            "xt": xt, "wq": wq, "wk": wk, "wv": wv, "wo": wo,
            "bq": bq, "bk": bk, "bv": bv, "mk": masks,
        })
    return in_maps


def assemble_output(inputs, shards, cfg=FULL):
    c = _derived(cfg)
    residual = np.asarray(inputs["residual"], np.float32)
    b_O = np.asarray(inputs["b_O"], np.float32)
    # each shard is tile-major [n_pb, n_mc, 128, 512] -> [rows, d_model]
    rows, d_model = c["rows"], c["d_model"]
    flat = [np.asarray(s).astype(np.float32).transpose(0, 2, 1, 3)
            .reshape(rows, d_model) for s in shards]
    out = np.concatenate(flat, axis=0)
    out = out.reshape(c["batch"], c["seq"], c["d_model"]) + b_O
    return residual, out.astype(np.float32)


_NC_CACHE = {}


def _get_nc():
    if "nc" not in _NC_CACHE:
        _NC_CACHE["nc"] = build_graph(FULL)
    return _NC_CACHE["nc"]


def run(inputs, trace=False):
    nc = _get_nc()
    in_maps = make_in_maps(inputs, FULL)
    try:
        res = run_bass_kernel_spmd(nc, in_maps, list(range(FULL["n_cores"])),
                                   trace=trace)
    except Exception:
        # a previous bad run can leave the remote device wedged for one
        # attempt; give it a moment and retry once
        import time
        time.sleep(60)
        res = run_bass_kernel_spmd(nc, in_maps, list(range(FULL["n_cores"])),
                                   trace=trace)
    shards = [res.results[i]["out"] for i in range(FULL["n_cores"])]
    residual, out = assemble_output(inputs, shards, FULL)
    return (residual, out), res


def kernel(**inputs):
    (residual, out), _ = run(inputs, trace=False)
    return (residual, out)



# revision 39
# speedup vs baseline: 1.0569x; 1.0203x over previous
"""Distributed multi-head causal attention for 8 TRN2 NeuronCores.

Problem: residual [2, 2048, 2048] f32 -> (residual, attn_out [2, 2048, 2048])
  q/k/v = residual @ W_{Q,K,V} + b  (16 heads, d_head 128)
  scores = q k^T / sqrt(128), causal mask, softmax
  out = (pattern @ v) @ W_O + b_O

Sharding: tensor-parallel over heads. Core c computes QKV projections and
attention for heads 2c, 2c+1 over both batches, producing z^T (the
pre-output-projection activations). Two 8-core AllToAlls (one per local
head) redistribute z^T from head-sharded to position-sharded: shard j covers
positions [512*j, 512*(j+1)) of the flattened [batch*seq] axis. After the
A2A each core holds all 16 heads for its own 512 positions and computes the
output projection for just those rows. The host concatenates the 8 shards.

All matmuls in bf16 (inputs pre-cast and pre-packed on host so every load is
one large contiguous DMA), accumulation f32 in PSUM:
  Q^T/K^T [dh, pos] = W^T X^T    (lhsT = W [model, dh], rhs = X^T)
  V [pos, dh*hpc]   = X W_V      (lhsT = X^T tile,      rhs = W_V heads)
  S^T [k, q]        = K Q^T      (lhsT = K^T tile,      rhs = Q^T)
  z^T [dh, q]       = V^T P^T    (lhsT = V tile,        rhs = P^T = exp(S^T))
  den [*, q]        = J acc      (lhsT = all-ones,      rhs = sum_k P^T)
  out [pos, m]      = z W_O      (lhsT = z^T tile,      rhs = W_O)

The emission driver software-pipelines the phases on the single PE queue:
attention chunks (h, b, qc) become runnable as soon as phase 1 has produced
q/k/v for (b, <=qc) (causality: chunk qc only attends keys < (qc+1)*512), so
attention matmuls interleave with later projection units and the attention
phase's scalar-engine exps and vector-engine denominator accumulations hide
under projection PE work. The AV pass is one full-width [128, 512] matmul
per k-tile; softmax denominators come from vector adds of the exp panels
(acc = sum_k P^T) reduced+broadcast by a single all-ones matmul per chunk.
Pass E of the output projection (first-A2A heads) starts the moment the last
attention matmul retires; pass O adds the second-A2A heads.

Schedule refinements over the first working version: output stores are
tile-major contiguous 128KB writes (host reassembles); W_O chunks 0/1 load
into a dedicated static (no xt-slot WAR); zf h1 loads spread over all three
DMA rings; pass-E staging aliases kt's slot and W_O chunk 3 aliases v's
slot; the PE warmup extends through the DMA-bound start (16x512 + 24x256
dummies) so the p-state never drops before the first real matmul.

Measured no-gos (do not retry): fp8 double-pump matmuls (any fp8 operand in
the chain gives ~3.5%+ output error vs the 2e-2 budget; z is itself an
average of random v's so quantization noise does not wash out); splitting
weight loads below 4KB-per-partition segments (ring BW drops ~2x); bulk
transfers on the gpsimd software-DGE ring; W_O loads queued ahead of the h1
ships in the sync ring FIFO (delays the second A2A trigger ~15us); zf h0
loads on gpsimd between the A2A triggers (cc0's rendezvous wait can outlast
the h1 ships under peer skew and then delays cc1); splitting the second A2A
into position-halves (halves the ship DMA segment size, slower overall);
PSUM st=4/z=2 rebalance (zp rotation convoys the attention chunks).
"""

import numpy as np
import ml_dtypes

import concourse.bass as bass
import concourse.tile as tile
from concourse import bacc, mybir
from concourse.bass_utils import run_bass_kernel_spmd
from concourse.tile_rust import add_dep_helper

BF16 = mybir.dt.bfloat16
F32 = mybir.dt.float32
NP_BF16 = ml_dtypes.bfloat16

FULL = dict(n_heads=16, d_model=2048, d_head=128, batch=2, seq=2048, n_cores=8)
ATTN_SCALE = float(np.sqrt(128.0))


def _derived(cfg):
    d = dict(cfg)
    d["hpc"] = d["n_heads"] // d["n_cores"]             # heads per core
    d["rows"] = d["batch"] * d["seq"] // d["n_cores"]   # out rows per core
    d["qc_size"] = d["rows"]                            # q-chunk == A2A shard
    assert d["qc_size"] <= 512
    d["n_qc"] = d["seq"] // d["qc_size"]                # q chunks per batch
    d["n_kb"] = d["seq"] // 128                         # k blocks per batch
    d["n_mb"] = d["d_model"] // 128                     # model-dim blocks
    d["n_dg"] = d["qc_size"] // 128                     # diag offsets per chunk
    d["n_mc"] = d["d_model"] // 512                     # out m-chunks
    d["n_pb"] = d["rows"] // 128                        # out pos-blocks
    assert d["n_qc"] * d["batch"] == d["n_cores"]
    assert d["d_head"] == 128
    return d


def build_graph(cfg=FULL, enable_asserts=False):
    c = _derived(cfg)
    hpc, QC = c["hpc"], c["qc_size"]
    n_qc, n_kb, n_mb, n_dg = c["n_qc"], c["n_kb"], c["n_mb"], c["n_dg"]
    n_mc, n_pb, rows = c["n_mc"], c["n_pb"], c["rows"]
    n_heads, d_model, seq = c["n_heads"], c["d_model"], c["seq"]
    batch, n_cores = c["batch"], c["n_cores"]
    dpb = QC // 128
    MC = 512

    nc = bacc.Bacc("TRN2", target_bir_lowering=False, debug=False,
                   enable_asserts=enable_asserts, num_devices=n_cores)

    # all inputs pre-packed on host into [128, ...] partition-major layouts
    xt_d = nc.dram_tensor("xt", [128, batch, n_qc, n_mb, QC], BF16,
                          kind="ExternalInput")
    wq_d = nc.dram_tensor("wq", [128, hpc, n_mb, 128], BF16, kind="ExternalInput")
    wk_d = nc.dram_tensor("wk", [128, hpc, n_mb, 128], BF16, kind="ExternalInput")
    wv_d = nc.dram_tensor("wv", [128, n_mb, hpc * 128], BF16, kind="ExternalInput")
    # chunk-major so each 2MB W_O chunk load is fully contiguous
    wo_d = nc.dram_tensor("wo", [n_mc, 128, n_heads, MC], BF16,
                          kind="ExternalInput")
    bq_d = nc.dram_tensor("bq", [128, hpc], F32, kind="ExternalInput")
    bk_d = nc.dram_tensor("bk", [128, hpc], F32, kind="ExternalInput")
    bv_d = nc.dram_tensor("bv", [hpc * 128], F32, kind="ExternalInput")
    mk_d = nc.dram_tensor("mk", [128, n_dg, QC], BF16, kind="ExternalInput")
    # bf16 output halves the final DMA tail; the host adds b_O in f32.
    # tile-major layout so every output store is one contiguous 128KB write
    # (row-major [rows, d_model] made the final DMAs 1KB-segment strided and
    # the kernel-end barrier waited ~10us for them to drain)
    out_d = nc.dram_tensor("out", [n_pb, n_mc, 128, MC], BF16,
                           kind="ExternalOutput")

    rg = [list(range(n_cores))]
    Exp = mybir.ActivationFunctionType.Exp

    with tile.TileContext(nc) as tc:
        with (
            tc.tile_pool(name="stat", bufs=1) as stat,
            tc.tile_pool(name="xin", bufs=2) as xin,
            tc.tile_pool(name="work", bufs=3) as work,
            tc.tile_pool(name="ps", bufs=2, space="PSUM") as ps,
            tc.tile_pool(name="dram", bufs=1, space="DRAM") as dram,
        ):
            wq_sb = stat.tile([128, hpc, n_mb, 128], BF16)
            wk_sb = stat.tile([128, hpc, n_mb, 128], BF16)
            wv_sb = stat.tile([128, n_mb, hpc * 128], BF16, tag="wvzf")
            qt_sb = stat.tile([128, batch, hpc, seq], BF16)
            kt_sb = stat.tile([128, batch, hpc, seq], BF16)
            v_sb = stat.tile([128, batch, n_kb, hpc, 128], BF16)
            # W_O chunks 0/1 get a dedicated static so their 2MB loads can
            # ride the sync ring mid-phase-1 (it idles once the xt stream
            # ends) instead of queueing behind the last attention ships
            wo01_sb = stat.tile([128, 2, n_heads, MC], BF16)
            bq_sb = stat.tile([128, hpc], F32)
            bk_sb = stat.tile([128, hpc], F32)
            vb_sb = stat.tile([128, hpc * 128], F32)
            mk_sb = stat.tile([128, n_dg, QC], BF16)
            ones_sb = stat.tile([128, 128], BF16)
            # h-major so each A2A's zf tiles land with ONE strided DMA
            zf_sb = stat.tile([128, hpc, n_cores, rows], BF16, tag="wvzf")

            a2a_in = [dram.tile([n_cores, 128, rows], BF16, name=f"a2ai{h}")
                      for h in range(hpc)]
            a2a_out = [dram.tile([n_cores, 128, rows], BF16, name=f"a2ao{h}")
                       for h in range(hpc)]

            # warmup memsets lead the vector queue so the dummy matmuls can
            # start the p-state ramp immediately after the engine barrier
            grb_sb = stat.tile([128, QC], BF16)
            nc.vector.memset(ones_sb, 1.0)
            nc.vector.memset(grb_sb[0:1, 0:1], 1.0)

            # startup loads spread over the three DMA-capable rings
            # (scalar/sync/gpsimd) in exact first-consumption order; wq h0
            # and wv split in halves so the first chains start as soon as
            # their leading mb blocks land instead of waiting for whole
            # weight tensors
            # big weights all on the scalar HWDGE queue as single descriptors
            # (issue cost ~0.6us each), ordered to match first consumption
            # (Q h0, K h0, V, Q h1, K h1); small consts on gpsimd; xt chunks
            # stream on sync alone so chunk N+1 is never stuck behind a
            # weight load. NOTE: splitting the weight loads into halves was
            # tried (v4) and is SLOWER — it halves the strided DMA segment
            # size (4KB -> 2KB) and ring bandwidth drops more than the
            # earlier start gains.
            xt00 = xin.tile([128, n_mb, QC], BF16, tag="xt", name="xt0_0")
            nc.scalar.dma_start(out=wq_sb[:, 0], in_=wq_d[:, 0])
            nc.sync.dma_start(out=xt00[:, 0:4], in_=xt_d[:, 0, 0, 0:4])
            nc.scalar.dma_start(out=xt00[:, 4:8], in_=xt_d[:, 0, 0, 4:8])
            nc.sync.dma_start(out=xt00[:, 8:12], in_=xt_d[:, 0, 0, 8:12])
            nc.scalar.dma_start(out=wv_sb, in_=wv_d[:])
            nc.sync.dma_start(out=xt00[:, 12:16], in_=xt_d[:, 0, 0, 12:16])
            for h in range(1, hpc):
                nc.scalar.dma_start(out=wq_sb[:, h], in_=wq_d[:, h])
            nc.gpsimd.dma_start(out=bq_sb, in_=bq_d[:])
            nc.gpsimd.dma_start(out=bk_sb, in_=bk_d[:])
            # K weights ride the otherwise-idle gpsimd ring so the first K
            # chains never queue behind the Q/V loads on scalar
            nc.gpsimd.dma_start(out=wk_sb[:, 0], in_=wk_d[:, 0])
            nc.gpsimd.dma_start(out=wk_sb[:, 1], in_=wk_d[:, 1])
            bv_ap = bv_d.ap()
            nc.gpsimd.dma_start(
                out=vb_sb,
                in_=bass.AP(tensor=bv_ap.tensor, offset=bv_ap.offset,
                            ap=[[0, 128]] + list(bv_ap.ap)))
            nc.gpsimd.dma_start(out=mk_sb, in_=mk_d[:])

            # PE warmup: full-width dummy matmuls bridge the gap between
            # the engine barrier and the first weight arrival so the p-state
            # ramp (needs ~3us of continuous HIGH-utilization busy to reach
            # max clock) completes on dummy work. The tail of the warmup is
            # 256-row dummies: fine-grained so a late-arriving first weight
            # is delayed at most ~130ns, but enough of them to keep the
            # clock ramped through the ~+21us DMA-bound start.
            for w in range(16):
                dmy = ps.tile([128, QC], F32, tag="A", name=f"wrm{w}")
                nc.tensor.matmul(dmy, ones_sb, grb_sb,
                                 start=True, stop=True)
            for w in range(24):
                dmy = ps.tile([128, QC // 2], F32, tag="A",
                              name=f"wrs{w}")
                nc.tensor.matmul(dmy, ones_sb, grb_sb[:, 0:QC // 2],
                                 start=True, stop=True)

            wo_tiles = {}
            last_exp = [None]
            markers = set()
            ship_n = [0] * hpc

            def emit_cc(h):
                nc.gpsimd.collective_compute(
                    "AllToAll", mybir.AluOpType.bypass, replica_groups=rg,
                    ins=[a2a_in[h].opt()], outs=[a2a_out[h].opt()])


            def load_zf(h, engs):
                # 8 fully-contiguous 128KB loads (a strided single-descriptor
                # variant measured 55GB/s vs ~350 contiguous), spread across
                # the given rings
                for i in range(n_cores):
                    eng = engs[i % len(engs)]
                    d = eng.dma_start(out=zf_sb[:, h, i, :],
                                      in_=a2a_out[h][i])
                    if eng is nc.scalar and last_exp[0] is not None:
                        # scalar ring also runs the exps: pin after the final
                        # exp so the scheduler cannot hoist the collective-
                        # completion wait into the middle of the attention
                        add_dep_helper(d.ins, last_exp[0].ins,
                                       reason="zf load after attention exps")

            # ---- stream 1: phase-1 QKV projection units ----
            # group order [Q h0, K h0, V, Q h1, K h1]: the ("v", b, qc)
            # marker frees head-0 attention chunks 2 groups early, pulling
            # the first A2A trigger ahead of the attention tail
            def p1_qk(b, qc, h, xt_c, ql):
                for (w_sb, b_sb, dst) in ((wq_sb, bq_sb, qt_sb),
                                          (wk_sb, bk_sb, kt_sb)):
                    pp = ps.tile([128, QC], F32, tag="A",
                                 name=f"psp{b}_{qc}_{h}")
                    for mb in range(n_mb):
                        nc.tensor.matmul(pp, w_sb[:, h, mb, :],
                                         xt_c[:, mb, :],
                                         start=(mb == 0),
                                         stop=(mb == n_mb - 1))
                    nc.vector.tensor_scalar_add(
                        dst[:, b, h, ql], pp, b_sb[:, h:h + 1])
                    yield n_mb * QC

            def p1_gen():
                deferred = []
                # b-alternating unit order spreads attention-chunk
                # availability (and so the attention phase's vector/scalar
                # work) evenly across the whole projection phase
                for qc in range(n_qc):
                    for b in range(batch):
                        ql = slice(qc * QC, (qc + 1) * QC)
                        if qc == 0 and b == 0:
                            xt_c = xt00  # loaded with the consts above
                        else:
                            xt_c = xin.tile([128, n_mb, QC], BF16, tag="xt",
                                            name=f"xt{b}_{qc}")
                            nc.sync.dma_start(out=xt_c, in_=xt_d[:, b, qc])
                        yield from p1_qk(b, qc, 0, xt_c, ql)
                        for pb4 in range(dpb):
                            pb = qc * dpb + pb4
                            pp = ps.tile([128, hpc * 128], F32, tag="A",
                                         name=f"psv{b}_{pb}")
                            for mb in range(n_mb):
                                nc.tensor.matmul(
                                    pp,
                                    xt_c[:, mb, pb4 * 128:(pb4 + 1) * 128],
                                    wv_sb[:, mb, :],
                                    start=(mb == 0), stop=(mb == n_mb - 1))
                            nc.vector.tensor_add(
                                v_sb[:, b, pb, :, :],
                                pp.rearrange("p (h d) -> p h d", h=hpc),
                                vb_sb.rearrange("p (h d) -> p h d", h=hpc))
                            yield n_mb * hpc * 128
                        markers.add(("v", b, qc))
                        if qc == n_qc - 1:
                            # defer the final units' h1 Q/K groups: the qc3
                            # head-0 attention chunks (which gate the first
                            # A2A trigger) unblock ~16us earlier, and these
                            # groups become extra post-trigger PE runway
                            deferred.append((b, qc, xt_c, ql))
                        else:
                            for h in range(1, hpc):
                                yield from p1_qk(b, qc, h, xt_c, ql)
                            markers.add(("full", b, qc))
                for b, qc, xt_c, ql in deferred:
                    for h in range(1, hpc):
                        yield from p1_qk(b, qc, h, xt_c, ql)
                    markers.add(("full", b, qc))


            # ---- stream 2: attention chunks, self-pipelined over k-tiles ----
            # qc-major order tracks unit availability; all h1 chunks of
            # qc1..3 are deferred behind the h0 chunks so the first A2A
            # trigger has ~43us of attention PE work after it — enough to
            # cover its rendezvous skew + transfer AND the CC-engine
            # serialization before the second A2A
            chunks = [(0, b, qc) for qc in range(n_qc) for b in range(batch)]
            chunks += [(1, b, qc) for qc in range(n_qc) for b in range(batch)]
            LAG = 2
            pend = []
            chain = {}

            def emit_score(ci, h, b, qc, kb, panel):
                dg = kb - qc * dpb
                off = 128 * dg if dg > 0 else 0
                np_ = QC - off
                st = ps.tile([128, QC], F32, tag="st", bufs=3,
                             name=f"st{ci}_{kb}")
                nc.tensor.matmul(st[:, :np_],
                                 kt_sb[:, b, h, kb * 128:(kb + 1) * 128],
                                 qt_sb[:, b, h, qc * QC + off:(qc + 1) * QC],
                                 start=True, stop=True)
                last_exp[0] = nc.scalar.activation(panel[:, kb, off:],
                                                   st[:, :np_], Exp)
                if dg >= 0:
                    band = slice(off, off + 128)
                    nc.vector.tensor_mul(panel[:, kb, band],
                                         panel[:, kb, band],
                                         mk_sb[:, dg, band])
                return np_

            def emit_av(ci, h, b, qc, kb, n_b, panel, zp, acc):
                dg = kb - qc * dpb
                off = 128 * dg if dg > 0 else 0
                nc.tensor.matmul(zp[:, off:], v_sb[:, b, kb, h, :],
                                 panel[:, kb, off:],
                                 start=(kb == 0), stop=(kb == n_b - 1),
                                 skip_group_check=True)
                # denominator partials: acc = sum_kb exp panel (masked rows
                # of diag tiles contribute only their valid [off:] columns).
                # bf16 accumulation: ~0.1% rms rounding per add, well inside
                # the error budget, and 16-bit DVE ops run ~1.7x faster
                if kb == 0:
                    nc.vector.tensor_copy(acc, panel[:, 0, :])
                else:
                    nc.vector.tensor_add(acc[:, off:], acc[:, off:],
                                         panel[:, kb, off:])
                return QC - off

            def emit_ship(ci):
                # reduce+broadcast the denominators with one all-ones matmul,
                # then normalize the full z psum into the A2A staging tile.
                # dps alternates psum tags so tail ship chains don't convoy
                # on a 2-bank rotation
                h, b, qc, zp, acc = chain.pop(ci)
                tg, bf = (("A", 2), ("st", 3))[ci % 2]
                dps = ps.tile([128, QC], F32, tag=tg, bufs=bf,
                              name=f"dps{ci}")
                nc.tensor.matmul(dps, ones_sb, acc, start=True, stop=True)
                rb = work.tile([128, QC], F32, tag="rb", bufs=2, name=f"rb{ci}")
                nc.vector.reciprocal_approx_fast(out=rb, in_=dps)
                zsb = work.tile([128, QC], BF16, tag="zsb", bufs=2,
                                name=f"zsb{ci}")
                nc.vector.tensor_mul(zsb, zp, rb)
                shard = b * n_qc + qc
                nc.sync.dma_start(out=a2a_in[h][shard], in_=zsb)
                ship_n[h] += 1
                if ship_n[h] == batch * n_qc:
                    emit_cc(h)
                return QC

            def attn_gen():
                for ci, (h, b, qc) in enumerate(chunks):
                    gate = ("v", b, qc) if h == 0 else ("full", b, qc)
                    while gate not in markers:
                        yield ("blocked", gate)
                    n_b = (qc + 1) * dpb
                    panel = work.tile([128, n_kb, QC], BF16, tag="pt", bufs=2,
                                      name=f"pt{ci}")
                    zp = ps.tile([128, QC], F32, tag="z", bufs=3,
                                 name=f"zp{ci}")
                    acc = work.tile([128, QC], BF16, tag="acc", bufs=2,
                                    name=f"acc{ci}")
                    for k in range(n_b + LAG):
                        r = 0
                        if k < n_b:
                            r += emit_score(ci, h, b, qc, k, panel)
                        if 0 <= k - LAG < n_b:
                            r += emit_av(ci, h, b, qc, k - LAG, n_b, panel,
                                         zp, acc)
                        if k == 3 and pend:
                            r += emit_ship(pend.pop(0))
                        yield r
                    chain[ci] = (h, b, qc, zp, acc)
                    pend.append(ci)
                while pend:
                    yield emit_ship(pend.pop(0))
                # W_O chunks 0/1 after the last ship on sync: any earlier
                # and their 4MB of transfers sit AHEAD of the h1 ships in
                # the ring FIFO, delaying the second A2A trigger ~15us
                # (measured). The dedicated static avoids the xt-slot WAR.
                for mc in range(min(2, n_mc)):
                    nc.sync.dma_start(out=wo01_sb[:, mc], in_=wo_d[mc])
                    wo_tiles[mc] = wo01_sb[:, mc]
                # first A2A's zf tiles lead the scalar ring's post-attention
                # work (the gpsimd software-DGE ring would stall the second
                # A2A trigger behind their transfers)
                load_zf(0, (nc.scalar,))

            # ---- emission driver: fraction-paced interleave ----
            def drive(specs):
                gens = [g for g, _ in specs]
                totals = [float(t) for _, t in specs]
                spent = [0.0] * len(specs)
                alive = [True] * len(specs)
                blocked = [None] * len(specs)
                while any(alive):
                    cands = [i for i in range(len(specs)) if alive[i] and
                             (blocked[i] is None or blocked[i] in markers)]
                    assert cands, "emission driver deadlock"
                    i = min(cands, key=lambda j: spent[j] / totals[j])
                    blocked[i] = None
                    try:
                        item = next(gens[i])
                    except StopIteration:
                        alive[i] = False
                        continue
                    if isinstance(item, tuple):
                        blocked[i] = item[1]
                    else:
                        spent[i] += item

            p1_rows = batch * n_qc * (2 * hpc * n_mb * QC
                                      + dpb * n_mb * hpc * 128)
            at_rows = 0
            for (h, b, qc) in chunks:
                n_b = (qc + 1) * dpb
                for kb in range(n_b):
                    dg = kb - qc * dpb
                    off = 128 * dg if dg > 0 else 0
                    at_rows += 2 * (QC - off)
                at_rows += QC
            drive([(p1_gen(), p1_rows), (attn_gen(), at_rows)])

            # scalar-queue order at attention end: W_O chunks 2/3 into the
            # released qt/v slots (no collective wait; pass O needs them
            # mid-pass), then zf odds (which wait on cc1). Pinned after the
            # exps so the scheduler cannot hoist the waits into attention.
            for mc, tg in zip(range(2, n_mc), ("qt_sb", "v_sb")):
                t = stat.tile([128, n_heads, MC], BF16, tag=tg, name=f"wo{mc}")
                d = nc.scalar.dma_start(out=t, in_=wo_d[mc])
                if last_exp[0] is not None:
                    add_dep_helper(d.ins, last_exp[0].ins,
                                   reason="wo load after attention exps")
                wo_tiles[mc] = t
            for h in range(1, hpc):
                # gpsimd ring is idle after the second A2A trigger, and the
                # trigger precedes these loads there by construction — so
                # the cc1-completion wait can't block anything else
                # three rings: post-attention the sync queue only has the
                # pass-O output stores left, and those start after cc1
                # completes anyway, so a cc1 wait on sync blocks nothing
                load_zf(h, (nc.gpsimd, nc.scalar, nc.sync))

            # ---- phase 3: output projection, two passes ----
            # Pass E runs the first-A2A heads for ALL output tiles staged to
            # SBUF in bf16; pass O adds the remaining heads once cc1's zf
            # tiles have arrived. (global head g = i*hpc + h for source
            # core i, local head h; zf is [128, h, i, rows])
            late_hi = [(h, i) for h in range(1, hpc) for i in range(n_cores)]
            # pass-E staging reuses kt's slot (dead after the last score MM)
            osbe = stat.tile([128, n_mc * n_pb, MC], BF16, tag="kt_sb")
            tiles3 = [(mc, pb) for mc in range(n_mc) for pb in range(n_pb)]
            for n, (mc, pb) in enumerate(tiles3):
                tg, bf = (("A", 2), ("st", 3))[n % 2]
                pp = ps.tile([128, MC], F32, tag=tg, bufs=bf,
                             name=f"pse{mc}_{pb}")
                for i in range(n_cores):
                    nc.tensor.matmul(
                        pp, zf_sb[:, 0, i, pb * 128:(pb + 1) * 128],
                        wo_tiles[mc][:, i * hpc, :],
                        start=(i == 0), stop=(i == n_cores - 1))
                nc.vector.tensor_copy(osbe[:, n, :], pp)
            for n, (mc, pb) in enumerate(tiles3):
                tg, bf = (("A", 2), ("st", 3))[n % 2]
                pp = ps.tile([128, MC], F32, tag=tg, bufs=bf,
                             name=f"pso{mc}_{pb}")
                for j, (h, i) in enumerate(late_hi):
                    nc.tensor.matmul(
                        pp, zf_sb[:, h, i, pb * 128:(pb + 1) * 128],
                        wo_tiles[mc][:, i * hpc + h, :],
                        start=(j == 0), stop=(j == len(late_hi) - 1))
                osb = work.tile([128, MC], BF16, tag="osb", bufs=2,
                                name=f"osb{mc}_{pb}")
                nc.vector.tensor_add(osb, pp, osbe[:, n, :])
                # alternate output rings so the final flush isn't serialized
                oeng = nc.sync if n % 2 == 0 else nc.scalar
                oeng.dma_start(out=out_d[pb, mc], in_=osb)

    nc.compile()
    return nc


def make_in_maps(inputs, cfg=FULL):
    c = _derived(cfg)
    hpc, QC = c["hpc"], c["qc_size"]
    n_mb, n_dg = c["n_mb"], c["n_dg"]
    d_model, seq, batch = c["d_model"], c["seq"], c["batch"]
    residual = np.asarray(inputs["residual"], np.float32)
    W_Q = np.asarray(inputs["W_Q"], np.float32)
    W_K = np.asarray(inputs["W_K"], np.float32)
    W_V = np.asarray(inputs["W_V"], np.float32)
    W_O = np.asarray(inputs["W_O"], np.float32)
    b_Q = np.asarray(inputs["b_Q"], np.float32)
    b_K = np.asarray(inputs["b_K"], np.float32)
    b_V = np.asarray(inputs["b_V"], np.float32)
    scale = 1.0 / ATTN_SCALE

    # X^T packed per q-chunk [128, batch, n_qc, n_mb, QC]:
    # [p, b, qc, mb, s'] = residual[b, qc*QC+s', mb*128+p]
    n_qc = c["n_qc"]
    xt = np.ascontiguousarray(
        residual.reshape(batch, n_qc, QC, n_mb, 128).transpose(4, 0, 1, 3, 2)
    ).astype(NP_BF16)
    # W_O packed chunk-major [n_mc, 128, n_heads, 512]:
    # [mc, p, g, m'] = W_O[g, p, mc*512+m']
    n_mc = c["n_mc"]
    wo = np.ascontiguousarray(
        W_O.transpose(1, 0, 2).reshape(128, c["n_heads"], n_mc, 512)
        .transpose(2, 0, 1, 3)).astype(NP_BF16)
    # causal {0,1} masks packed [128, n_dg, QC]
    masks = np.zeros((128, n_dg, QC), np.float32)
    pk = np.arange(128)[:, None]
    fq = np.arange(QC)[None, :]
    for dg in range(n_dg):
        masks[:, dg, :] = (fq >= pk + 128 * dg).astype(np.float32)
    masks = masks.astype(NP_BF16)

    in_maps = []
    for core in range(c["n_cores"]):
        hs = slice(core * hpc, (core + 1) * hpc)
        # [128, hpc, n_mb, 128]: [p, h, mb, d] = W[h, mb*128+p, d]
        wq = np.ascontiguousarray(
            (W_Q[hs] * scale).reshape(hpc, n_mb, 128, 128).transpose(2, 0, 1, 3)
        ).astype(NP_BF16)
        wk = np.ascontiguousarray(
            W_K[hs].reshape(hpc, n_mb, 128, 128).transpose(2, 0, 1, 3)
        ).astype(NP_BF16)
        # [128, n_mb, hpc*128]: [p, mb, (h d)] = W_V[h, mb*128+p, d]
        wv = np.ascontiguousarray(
            W_V[hs].reshape(hpc, n_mb, 128, 128).transpose(2, 1, 0, 3)
            .reshape(128, n_mb, hpc * 128)).astype(NP_BF16)
        bq = np.ascontiguousarray((b_Q[hs] * scale).T).astype(np.float32)
        bk = np.ascontiguousarray(b_K[hs].T).astype(np.float32)
        bv = np.ascontiguousarray(b_V[hs].reshape(hpc * 128)).astype(np.float32)
        in_maps.append({
            "xt": xt, "wq": wq, "wk": wk, "wv": wv, "wo": wo,
            "bq": bq, "bk": bk, "bv": bv, "mk": masks,
        })
    return in_maps


def assemble_output(inputs, shards, cfg=FULL):
    c = _derived(cfg)
    residual = np.asarray(inputs["residual"], np.float32)
    b_O = np.asarray(inputs["b_O"], np.float32)
    # each shard is tile-major [n_pb, n_mc, 128, 512] -> [rows, d_model]
    rows, d_model = c["rows"], c["d_model"]
    flat = [np.asarray(s).astype(np.float32).transpose(0, 2, 1, 3)
            .reshape(rows, d_model) for s in shards]
    out = np.concatenate(flat, axis=0)
    out = out.reshape(c["batch"], c["seq"], c["d_model"]) + b_O
    return residual, out.astype(np.float32)


_NC_CACHE = {}


def _get_nc():
    if "nc" not in _NC_CACHE:
        _NC_CACHE["nc"] = build_graph(FULL)
    return _NC_CACHE["nc"]


def run(inputs, trace=False):
    nc = _get_nc()
    in_maps = make_in_maps(inputs, FULL)
    try:
        res = run_bass_kernel_spmd(nc, in_maps, list(range(FULL["n_cores"])),
                                   trace=trace)
    except Exception:
        # a previous bad run can leave the remote device wedged for one
        # attempt; give it a moment and retry once
        import time
        time.sleep(60)
        res = run_bass_kernel_spmd(nc, in_maps, list(range(FULL["n_cores"])),
                                   trace=trace)
    shards = [res.results[i]["out"] for i in range(FULL["n_cores"])]
    residual, out = assemble_output(inputs, shards, FULL)
    return (residual, out), res


def kernel(**inputs):
    (residual, out), _ = run(inputs, trace=False)
    return (residual, out)

